# revision 2
# baseline (speedup 1.0000x reference)
"""GQA (B=2, S=2048, d_model=2048, 16 Q heads / 4 KV groups) + output projection.

Sharding: 8 cores, core c <-> (b = c//4, g = c%4). Each core computes full
attention for the 4 query heads of KV group g of batch b, then multiplies its
512-feature slice of the concatenated head outputs with the matching 512 rows
of Wc^T, producing a partial [S, d_model] projection. Host sums the 4 partials
per batch element (bias is folded into the g==0 core's partial).

On-core layout: everything transposed.
  scoresT[t, s] = kT.T @ qT           (lhsT = kT tile [d,128t], rhs = qT [d,512s])
  expT = exp(scoresT / sqrt(128))     (ACT, fused scale, no max subtraction:
                                       scores ~ N(0,1), max ~6 over the tensor)
  sums[1, s]  = ones.T @ expT         (PE, accumulated over 16 t tiles)
  uT[hd, s]   = v.T @ expT            (PE, accumulated; v tile is [t,hd])
  attnT = uT * (1 / bcast(sums))      (PE k=1 ones broadcast + DVE recip/mult)
  out[s, o]   = attnT.T @ wT + bias   (PE, contraction over the 512 features)
All matmul operands are float32r (single-pass PE, ~1e-4 rel err per matmul).
"""

import math
import sys

sys.path.insert(0, "/opt/trn_rl_repo")

import numpy as np

import concourse.bacc as bacc
import concourse.bass as bass
import concourse.mybir as mybir
import concourse.tile as tile
from concourse.bass import ds, ts
from concourse.bass_utils import run_bass_kernel_spmd

F32 = mybir.dt.float32
F32R = mybir.dt.float32r

B = 2
S = 2048
D_MODEL = 2048
N_GROUPS = 4
HEADS_PER_GROUP = 4
HEAD_DIM = 128
P = 128
NT = S // P          # 16 t tiles
NJ = S // 512        # 4 s blocks
SCALE = 1.0 / math.sqrt(HEAD_DIM)

_COMPILED = None


def _build():
    nc = bacc.Bacc(None, target_bir_lowering=False)

    qT_d = nc.dram_tensor("qT", [P, HEADS_PER_GROUP, S], F32, kind="ExternalInput")
    kT_d = nc.dram_tensor("kT", [P, S], F32, kind="ExternalInput")
    v_d = nc.dram_tensor("v", [S, P], F32, kind="ExternalInput")
    wT_d = nc.dram_tensor("wT", [HEADS_PER_GROUP * P, D_MODEL], F32, kind="ExternalInput")
    bias_d = nc.dram_tensor("bias", [1, D_MODEL], F32, kind="ExternalInput")
    out_d = nc.dram_tensor("out", [S, D_MODEL], F32, kind="ExternalOutput")

    Exp = mybir.ActivationFunctionType.Exp
    mult = mybir.AluOpType.mult

    with tile.TileContext(nc) as tc:
        with (
            tc.tile_pool(name="const", bufs=1) as const_pool,
            tc.tile_pool(name="qt", bufs=3) as qt_pool,
            tc.tile_pool(name="expT", bufs=16) as expT_pool,
            tc.tile_pool(name="attnT", bufs=8) as attnT_pool,
            tc.tile_pool(name="small", bufs=2) as small_pool,
            tc.tile_pool(name="orow", bufs=2) as orow_pool,
            tc.tile_pool(name="qk_ps", bufs=2, space="PSUM") as qk_psum,
            tc.tile_pool(name="pv_ps", bufs=2, space="PSUM") as pv_psum,
            tc.tile_pool(name="misc_ps", bufs=1, space="PSUM") as misc_psum,
        ):
            ones_col_f = const_pool.tile([P, 1], F32, tag="ones_col_f")
            nc.vector.memset(ones_col_f[:], 1.0)
            ones_col = const_pool.tile([P, 1], F32R, tag="ones_col")
            nc.vector.tensor_copy(ones_col[:], ones_col_f[:])
            ones_row_f = const_pool.tile([1, P], F32, tag="ones_row_f")
            nc.vector.memset(ones_row_f[:], 1.0)
            ones_row = const_pool.tile([1, P], F32R, tag="ones_row")
            nc.vector.tensor_copy(ones_row[:], ones_row_f[:])

            kT_sb = const_pool.tile([P, S], F32R, tag="kT")
            nc.sync.dma_start(kT_sb[:], kT_d[:].bitcast(F32R))
            v_sb = const_pool.tile([P, NT, P], F32R, tag="v")
            nc.sync.dma_start(
                v_sb[:], v_d.rearrange("(n p) d -> p n d", p=P).bitcast(F32R)
            )
            bias_sb = const_pool.tile([1, D_MODEL], F32R, tag="bias")
            nc.sync.dma_start(bias_sb[:], bias_d[:].bitcast(F32R))
            wT_sb = const_pool.tile([P, HEADS_PER_GROUP, D_MODEL], F32R, tag="wT")
            nc.sync.dma_start(
                wT_sb[:], wT_d.rearrange("(n p) o -> p n o", p=P).bitcast(F32R)
            )

            attnT_tiles = {}

            def emit_qk(k):
                j, h = divmod(k, HEADS_PER_GROUP)
                qt = qt_pool.tile([P, 512], F32R, tag="qT")
                nc.sync.dma_start(qt[:], qT_d[:, h, ts(j, 512)].bitcast(F32R))
                pairs = []
                for pp in range(NT // 2):
                    ps = qk_psum.tile([P, 2, 512], F32, tag="qk")
                    et = expT_pool.tile([P, 2, 512], F32R, tag="expT")
                    for u in range(2):
                        tt = pp * 2 + u
                        nc.tensor.matmul(
                            ps[:, u, :], kT_sb[:, ts(tt, P)], qt[:],
                            start=True, stop=True,
                        )
                    nc.scalar.activation(et[:], ps[:], Exp, scale=SCALE)
                    pairs.append(et)
                return pairs

            def emit_sumpv(k, pairs):
                j, h = divmod(k, HEADS_PER_GROUP)
                sum_ps = misc_psum.tile([1, 512], F32, tag="sum")
                pv_ps = pv_psum.tile([P, 512], F32, tag="pv")
                for tt in range(NT):
                    et = pairs[tt // 2][:, tt % 2, :]
                    nc.tensor.matmul(
                        sum_ps[:], ones_col[:], et,
                        start=(tt == 0), stop=(tt == NT - 1),
                    )
                    nc.tensor.matmul(
                        pv_ps[:], v_sb[:, tt, :], et,
                        start=(tt == 0), stop=(tt == NT - 1),
                    )
                sum_sb = small_pool.tile([1, 512], F32R, tag="sumsb")
                nc.vector.tensor_copy(sum_sb[:], sum_ps[:])
                bc_ps = misc_psum.tile([P, 512], F32, tag="bcast")
                nc.tensor.matmul(bc_ps[:], ones_row[:], sum_sb[:], start=True, stop=True)
                rb_sb = small_pool.tile([P, 512], F32, tag="rb")
                nc.vector.reciprocal(rb_sb[:], bc_ps[:])
                at = attnT_pool.tile([P, 512], F32R, tag="attnT")
                nc.vector.tensor_tensor(at[:], pv_ps[:], rb_sb[:], mult)
                attnT_tiles[(j, h)] = at

            def emit_proj(j):
                for st in range(4):
                    orow = orow_pool.tile([P, D_MODEL], F32, tag="orow")
                    for ob in range(4):
                        po = pv_psum.tile([P, 512], F32, tag="pv")
                        for h in range(HEADS_PER_GROUP):
                            nc.tensor.matmul(
                                po[:], attnT_tiles[(j, h)][:, ts(st, P)],
                                wT_sb[:, h, ts(ob, 512)],
                                start=(h == 0), stop=False,
                            )
                        nc.tensor.matmul(
                            po[:], ones_row[:], bias_sb[:, ts(ob, 512)],
                            start=False, stop=True,
                        )
                        nc.vector.tensor_copy(orow[:, ts(ob, 512)], po[:])
                    nc.sync.dma_start(out_d[ds(j * 512 + st * P, P), :], orow[:])

            n_combos = NJ * HEADS_PER_GROUP
            prev = None
            for k in range(n_combos):
                pairs = emit_qk(k)
                if prev is not None:
                    emit_sumpv(k - 1, prev)
                    if (k - 1) % HEADS_PER_GROUP == HEADS_PER_GROUP - 1:
                        emit_proj((k - 1) // HEADS_PER_GROUP)
                prev = pairs
            emit_sumpv(n_combos - 1, prev)
            emit_proj(NJ - 1)

    nc.compile()
    return nc


def _get_nc():
    global _COMPILED
    if _COMPILED is None:
        _COMPILED = _build()
    return _COMPILED


def _shard_inputs(q, k, v, Wc, bc):
    in_maps = []
    for c in range(8):
        b, g = divmod(c, 4)
        qT = np.ascontiguousarray(
            q[b][:, g * 512:(g + 1) * 512].reshape(S, HEADS_PER_GROUP, P).transpose(2, 1, 0)
        )
        kT = np.ascontiguousarray(k[b][:, g * P:(g + 1) * P].T)
        vv = np.ascontiguousarray(v[b][:, g * P:(g + 1) * P])
        wT = np.ascontiguousarray(Wc[:, g * 512:(g + 1) * 512].T)
        if g == 0:
            bias = np.ascontiguousarray(bc.reshape(1, D_MODEL))
        else:
            bias = np.zeros((1, D_MODEL), dtype=np.float32)
        in_maps.append({"qT": qT, "kT": kT, "v": vv, "wT": wT, "bias": bias})
    return in_maps


def _run(inputs, trace=False):
    q = np.asarray(inputs["q"], dtype=np.float32)
    k = np.asarray(inputs["k"], dtype=np.float32)
    v = np.asarray(inputs["v"], dtype=np.float32)
    Wc = np.asarray(inputs["Wc"], dtype=np.float32)
    bc = np.asarray(inputs["bc"], dtype=np.float32)

    nc = _get_nc()
    in_maps = _shard_inputs(q, k, v, Wc, bc)
    res = run_bass_kernel_spmd(nc, in_maps, list(range(8)), trace=trace)

    out = np.empty((B, S, D_MODEL), dtype=np.float32)
    for b in range(B):
        acc = res.results[4 * b]["out"].astype(np.float32).copy()
        for g in range(1, 4):
            acc += res.results[4 * b + g]["out"]
        out[b] = acc
    return out, res


def kernel(**inputs):
    out, _ = _run(inputs, trace=False)
    return out
